# revision 1
# baseline (speedup 1.0000x reference)
"""Cosine-sim multi-head attention on 8 trn2 NeuronCores.

Sharding: core c -> (batch b = c//2, head-half hg = c%2). Each core computes
QKV projections for its 6 heads, full attention over S=2048, and a partial
out-projection [S, 768]. Host sums the two partials per batch and adds bo.

Per-core device layout (all matmul operands bf16, PSUM fp32):
  hst   [768, 2048]  hidden_states[b].T
  wqt/wkt/wvt [768, 384]  W[rows].T  (rows = hg*384 : hg*384+384)
  wot   [384, 768]   Wo[:, rows].T
  qT/kT/vT [384, 2048] computed transposed (pair p = m-tile = 2 heads)
  scoresT[j, i] per head; exp on ACT; PV with ones-augmented v -> ctx + denom.
"""
import numpy as np
import ml_dtypes

import concourse.bass as bass
import concourse.bacc as bacc
import concourse.tile as tile
from concourse import mybir

BF16 = mybir.dt.bfloat16
F32 = mybir.dt.float32
EXP = mybir.ActivationFunctionType.Exp
LN = mybir.ActivationFunctionType.Ln

B, S, D = 4, 2048, 768
H, DH = 12, 64
HPC = 6            # heads per core
NPAIR = 3          # head pairs per core (m-tiles of 128)
NJC = S // 128     # 16 j-chunks
NIC = S // 512     # 4 i-blocks
MAX_LOG_SCALE = float(np.log(1.0 / 0.01))

_NC_CACHE = {}


def build_nc():
    nc = bacc.Bacc(None, target_bir_lowering=False, debug=False)

    hst = nc.dram_tensor("hst", [D, S], BF16, kind="ExternalInput")
    wqt = nc.dram_tensor("wqt", [D, 384], BF16, kind="ExternalInput")
    wkt = nc.dram_tensor("wkt", [D, 384], BF16, kind="ExternalInput")
    wvt = nc.dram_tensor("wvt", [D, 384], BF16, kind="ExternalInput")
    wot = nc.dram_tensor("wot", [384, D], BF16, kind="ExternalInput")
    bq3 = nc.dram_tensor("bq3", [128, 3], F32, kind="ExternalInput")
    bk3 = nc.dram_tensor("bk3", [128, 3], F32, kind="ExternalInput")
    bv3 = nc.dram_tensor("bv3", [128, 3], F32, kind="ExternalInput")
    lns = nc.dram_tensor("lns", [128, 3], F32, kind="ExternalInput")
    i2d = nc.dram_tensor("i2d", [128, 2], BF16, kind="ExternalInput")
    o = nc.dram_tensor("o", [S, D], F32, kind="ExternalOutput")

    with tile.TileContext(nc) as tc:
        import contextlib
        with contextlib.ExitStack() as ctx:
            const = ctx.enter_context(tc.tile_pool(name="const", bufs=1))
            work = ctx.enter_context(tc.tile_pool(name="work", bufs=1, space="PSUM"))
            praw = ctx.enter_context(tc.tile_pool(name="praw", bufs=2))
            kraw_p = ctx.enter_context(tc.tile_pool(name="kraw", bufs=2))
            vtp = ctx.enter_context(tc.tile_pool(name="vtp", bufs=2))
            sqp = ctx.enter_context(tc.tile_pool(name="sqp", bufs=2))
            qsp = ctx.enter_context(tc.tile_pool(name="qsp", bufs=2))
            ksp = ctx.enter_context(tc.tile_pool(name="ksp", bufs=2))
            vap = ctx.enter_context(tc.tile_pool(name="vap", bufs=2))
            rnp = ctx.enter_context(tc.tile_pool(name="rnp", bufs=2))
            rrp = ctx.enter_context(tc.tile_pool(name="rrp", bufs=2))
            bcp = ctx.enter_context(tc.tile_pool(name="bcp", bufs=4))
            cnp = ctx.enter_context(tc.tile_pool(name="cnp", bufs=3))
            cup = ctx.enter_context(tc.tile_pool(name="cup", bufs=3))
            rdp = ctx.enter_context(tc.tile_pool(name="rdp", bufs=2))
            cnbp = ctx.enter_context(tc.tile_pool(name="cnbp", bufs=2))
            stgp = ctx.enter_context(tc.tile_pool(name="stgp", bufs=4))
            dram = ctx.enter_context(tc.tile_pool(name="dram", bufs=4, space="DRAM"))

            # ---- constants ----
            hst_sb = const.tile([128, 6, S], BF16)
            for c in range(6):
                eng = nc.sync if c % 2 == 0 else nc.scalar
                eng.dma_start(out=hst_sb[:, c, :], in_=hst[c * 128:(c + 1) * 128, :])
            w_sbs = []
            for name, wt in (("wq", wqt), ("wk", wkt), ("wv", wvt)):
                w_sb = const.tile([128, 6, 384], BF16, tag=name)
                for c in range(6):
                    eng = nc.scalar if c % 2 == 0 else nc.sync
                    eng.dma_start(out=w_sb[:, c, :], in_=wt[c * 128:(c + 1) * 128, :])
                w_sbs.append(w_sb)
            wot_sb = const.tile([128, 3, D], BF16)
            for c in range(3):
                nc.sync.dma_start(out=wot_sb[:, c, :], in_=wot[c * 128:(c + 1) * 128, :])
            b_sbs = []
            for name, bt in (("bq", bq3), ("bk", bk3), ("bv", bv3)):
                b_sb = const.tile([128, 3], F32, tag=name)
                nc.sync.dma_start(out=b_sb, in_=bt[:, :])
                b_sbs.append(b_sb)
            lns_sb = const.tile([128, 3], F32, tag="lns")
            nc.sync.dma_start(out=lns_sb, in_=lns[:, :])
            i2_sb = const.tile([128, 2], BF16, tag="i2")
            nc.sync.dma_start(out=i2_sb, in_=i2d[:, :])

            ctxns = []

            def qkv_and_norms(p):
                """Project pair p (m-tile p) of q/k/v; compute qs, ks, v_aug."""
                dests = []
                for ti, (w_sb, b_sb) in enumerate(zip(w_sbs, b_sbs)):
                    pool = (praw, kraw_p, vtp)[ti]
                    dest = pool.tile([128, S], BF16, tag=f"t{ti}")
                    for ib in range(4):
                        ps = work.tile([128, 512], F32, tag="work", bufs=2)
                        i0 = ib * 512
                        for kc in range(6):
                            nc.tensor.matmul(
                                ps,
                                w_sb[:, kc, p * 128:(p + 1) * 128],
                                hst_sb[:, kc, i0:i0 + 512],
                                start=(kc == 0), stop=(kc == 5))
                        nc.vector.tensor_scalar(
                            out=dest[:, i0:i0 + 512],
                            in0=ps,
                            scalar1=b_sb[:, p:p + 1],
                            scalar2=None,
                            op0=mybir.AluOpType.add)
                    dests.append(dest)
                qraw, kraw, vT = dests

                # norms^2 via block-ones matmul, M-packed by i-block into 2 banks
                rn_tiles = []
                for bank, src in ((0, qraw), (1, kraw)):
                    rn = work.tile([128, 512], F32, tag="work", bufs=2,
                                   name=f"rn{bank}")
                    nc.vector.memset(rn, 1.0)
                    for ib in range(4):
                        sq = sqp.tile([128, 512], BF16, tag="sq")
                        nc.vector.tensor_mul(sq, src[:, ib * 512:(ib + 1) * 512],
                                             src[:, ib * 512:(ib + 1) * 512])
                        nc.tensor.matmul(rn[32 * ib:32 * ib + 2, :],
                                         i2_sb, sq,
                                         start=True, stop=True,
                                         tile_position=(0, 32 * ib))
                    rn_tiles.append(rn)
                # rsqrt = exp(-0.5*ln(x)); q rows also add ln(scale_h)
                ln_sb = rnp.tile([128, 2, 512], F32, tag="ln")
                for bank in range(2):
                    nc.scalar.activation(ln_sb[:, bank, :], rn_tiles[bank], LN)
                rr = rrp.tile([128, 2, 512], BF16, tag="rr")
                nc.scalar.activation(rr[:, 0, :], ln_sb[:, 0, :], EXP,
                                     scale=-0.5, bias=lns_sb[:, p:p + 1])
                nc.scalar.activation(rr[:, 1, :], ln_sb[:, 1, :], EXP, scale=-0.5)

                # bounce rows to DRAM, broadcast to rq_bc / rk_bc
                rbs = []
                for bank in range(2):
                    r_dr = dram.tile([4, 2, 512], BF16, tag=f"rd{bank}")
                    for ib in range(4):
                        nc.sync.dma_start(out=r_dr[ib, :, :],
                                          in_=rr[32 * ib:32 * ib + 2, bank, :])
                    r_bc = bcp.tile([128, S], BF16, tag=f"rb{bank}")
                    for hh in range(2):
                        col = r_dr[:, hh, :]
                        src = bass.AP(tensor=col.tensor, offset=col.offset,
                                      ap=[[0, 64]] + col.ap)
                        nc.sync.dma_start(
                            out=r_bc[hh * 64:(hh + 1) * 64, :].rearrange(
                                "p (a b) -> p a b", a=4),
                            in_=src)
                    rbs.append(r_bc)

                qs = qsp.tile([128, S], BF16, tag="qs")
                nc.vector.tensor_mul(qs, qraw, rbs[0])
                ks = ksp.tile([128, S], BF16, tag="ks")
                nc.vector.tensor_mul(ks, kraw, rbs[1])

                return qs, ks, vT

            def build_va(vT):
                va = vap.tile([128, NJC, 130], BF16, tag="va")
                nc.vector.memset(va[:, :, 64:65], 1.0)
                nc.vector.memset(va[:, :, 129:130], 1.0)
                for c in range(NJC):
                    stga = stgp.tile([128, 64], BF16, tag="stga", name="stga")
                    stgb = stgp.tile([128, 64], BF16, tag="stgb", name="stgb")
                    nc.sync.dma_start_transpose(
                        out=stga, in_=vT[0:64, c * 128:(c + 1) * 128])
                    nc.sync.dma_start_transpose(
                        out=stgb, in_=vT[64:128, c * 128:(c + 1) * 128])
                    nc.vector.tensor_copy(va[:, c, 0:64], stga)
                    nc.vector.tensor_copy(va[:, c, 65:129], stgb)
                return va

            def attention(p, qs, ks, va, scores, epool, cpool, hooks=()):
                ctxn = cnp.tile([128, S], BF16, tag="ctxn")
                for ic in range(NIC):
                    for hic, fn in hooks:
                        if ic == hic:
                            fn()
                    i0 = ic * 512
                    ctxs = [cpool.tile([65, 512], F32, tag=f"ctx{hh}",
                                       name=f"ctx{hh}") for hh in range(2)]
                    for jc in range(NJC):
                        s_ps = scores.tile([128, 2, 512], F32, tag="s")
                        nc.tensor.matmul(s_ps[:, 0, :],
                                         ks[0:64, jc * 128:(jc + 1) * 128],
                                         qs[0:64, i0:i0 + 512],
                                         start=True, stop=True, tile_position=(0, 0))
                        nc.tensor.matmul(s_ps[:, 1, :],
                                         ks[64:128, jc * 128:(jc + 1) * 128],
                                         qs[64:128, i0:i0 + 512],
                                         start=True, stop=True, tile_position=(64, 0))
                        e_sb = epool.tile([128, 2, 512], BF16, tag="e")
                        nc.scalar.activation(e_sb.rearrange("p a b -> p (a b)"),
                                             s_ps.rearrange("p a b -> p (a b)"), EXP)
                        nc.tensor.matmul(ctxs[0], va[:, jc, 0:65], e_sb[:, 0, :],
                                         start=(jc == 0), stop=(jc == NJC - 1))
                        nc.tensor.matmul(ctxs[1], va[:, jc, 65:130], e_sb[:, 1, :],
                                         start=(jc == 0), stop=(jc == NJC - 1))
                    # drain psum: ctx (+denom row 64) -> sbuf bf16
                    ctxu = cup.tile([65, 2, 512], BF16, tag="cu")
                    for hh in range(2):
                        nc.vector.tensor_copy(ctxu[:, hh, :], ctxs[hh])
                    # rden = 1/denom, bounce + bcast, normalize
                    rden = rdp.tile([65, 2, 512], F32, tag="rden")
                    for hh in range(2):
                        nc.vector.reciprocal(rden[64:65, hh, :], ctxu[64:65, hh, :])
                    rd_dr = dram.tile([2, 512], F32, tag="rdd")
                    for hh in range(2):
                        nc.sync.dma_start(out=rd_dr[hh:hh + 1, :],
                                          in_=rden[64:65, hh, :])
                    for hh in range(2):
                        row = rd_dr[hh:hh + 1, :]
                        src = bass.AP(tensor=row.tensor, offset=row.offset,
                                      ap=[[0, 64]] + row.ap[1:])
                        rbc = bcp.tile([64, 512], F32, tag="rbc")
                        nc.sync.dma_start(out=rbc, in_=src)
                        if hh == 0:
                            nc.vector.tensor_mul(ctxn[0:64, i0:i0 + 512],
                                                 ctxu[0:64, 0, :], rbc)
                        else:
                            cnb = cnbp.tile([64, 512], BF16, tag="cnb")
                            nc.vector.tensor_mul(cnb, ctxu[0:64, 1, :], rbc)
                            nc.sync.dma_start(out=ctxn[64:128, i0:i0 + 512],
                                              in_=cnb)
                return ctxn

            with tc.tile_pool(name="scores", bufs=2, space="PSUM") as scores, \
                 tc.tile_pool(name="epool", bufs=6) as epool, \
                 tc.tile_pool(name="cpool", bufs=1, space="PSUM") as cpool:
                pending = {}
                qs0, ks0, vT0 = qkv_and_norms(0)
                pending[0] = (qs0, ks0, build_va(vT0))

                def make_hooks(pn):
                    part = {}

                    def h1():
                        part["qkv"] = qkv_and_norms(pn)

                    def h2():
                        qs_, ks_, vT_ = part["qkv"]
                        pending[pn] = (qs_, ks_, build_va(vT_))
                    return ((1, h1), (3, h2))

                for p in range(NPAIR):
                    qs, ks, va = pending.pop(p)
                    hooks = make_hooks(p + 1) if p + 1 < NPAIR else ()
                    ctxns.append(attention(p, qs, ks, va, scores, epool, cpool,
                                           hooks=hooks))

            # out-projection: 1-bank accumulation groups in the work slots,
            # DVE drains, emitted after the pair loop so the scheduler can
            # pull them into pair-2 attention slack.
            with tc.tile_pool(name="osb", bufs=3) as osb:
                for st in range(16):
                    o_sb = osb.tile([128, D], F32, tag="osb")
                    for nn in range(2):
                        o_ps = work.tile([128, 512], F32, tag="work", bufs=2,
                                         name="o_ps")
                        for p in range(NPAIR):
                            nc.tensor.matmul(
                                o_ps[:, 0:384],
                                ctxns[p][:, st * 128:(st + 1) * 128],
                                wot_sb[:, p, nn * 384:(nn + 1) * 384],
                                start=(p == 0), stop=(p == NPAIR - 1))
                        nc.vector.tensor_copy(o_sb[:, nn * 384:(nn + 1) * 384],
                                              o_ps[:, 0:384])
                    nc.sync.dma_start(out=o[st * 128:(st + 1) * 128, :], in_=o_sb)

    # Bias the ACT table-set choice: the greedy insert_act_table_loads pass
    # alternates natural_log <-> exp_and_others every pair (8 x ~2.7us table
    # swaps on the ACT critical path). Stripping Exp/Ln from the
    # single-function sets (dict order and indices preserved) forces the
    # combined natural_log_exp_and_others set -> one load total.
    import concourse.bacc as _bacc_mod
    real = _bacc_mod.get_activation_tables(nc.m.arch)
    patched = {}
    for name, fns in real.items():
        if name != "natural_log_exp_and_others":
            fns = {f for f in fns
                   if str(f).split(".")[-1] not in ("Exp", "Ln")}
        patched[name] = fns
    orig = _bacc_mod.get_activation_tables
    _bacc_mod.get_activation_tables = lambda arch: patched
    try:
        nc.compile()
    finally:
        _bacc_mod.get_activation_tables = orig
    return nc


def _prep_core_inputs(inputs, b, hg):
    bf = ml_dtypes.bfloat16
    hs = inputs["hidden_states"]
    rows = slice(hg * 384, (hg + 1) * 384)
    scale6 = np.exp(np.minimum(
        inputs["logit_scale"].reshape(H)[hg * HPC:(hg + 1) * HPC],
        MAX_LOG_SCALE)).astype(np.float64)

    def b3(bias):
        return np.ascontiguousarray(bias[rows].reshape(3, 128).T).astype(np.float32)

    lns = np.zeros((128, 3), np.float32)
    for p in range(3):
        for ib in range(4):
            for hh in range(2):
                lns[32 * ib + hh, p] = np.log(scale6[p * 2 + hh])
    i2 = np.zeros((128, 2), np.float32)
    i2[0:64, 0] = 1.0
    i2[64:128, 1] = 1.0
    return {
        "hst": np.ascontiguousarray(hs[b].T).astype(bf),
        "wqt": np.ascontiguousarray(inputs["Wq"][rows].T).astype(bf),
        "wkt": np.ascontiguousarray(inputs["Wk"][rows].T).astype(bf),
        "wvt": np.ascontiguousarray(inputs["Wv"][rows].T).astype(bf),
        "wot": np.ascontiguousarray(inputs["Wo"][:, rows].T).astype(bf),
        "bq3": b3(inputs["bq"]),
        "bk3": b3(inputs["bk"]),
        "bv3": b3(inputs["bv"]),
        "lns": lns,
        "i2d": i2.astype(bf),
    }


def kernel(**inputs):
    from concourse.bass_utils import run_bass_kernel_spmd
    inputs = {k: np.asarray(v) for k, v in inputs.items()}
    if "nc" not in _NC_CACHE:
        _NC_CACHE["nc"] = build_nc()
    nc = _NC_CACHE["nc"]
    in_maps = [_prep_core_inputs(inputs, c // 2, c % 2) for c in range(8)]
    res = run_bass_kernel_spmd(nc, in_maps, core_ids=list(range(8)))
    out = np.empty((B, S, D), np.float32)
    bo = inputs["bo"].astype(np.float32)
    for b in range(B):
        out[b] = res.results[2 * b]["o"] + res.results[2 * b + 1]["o"] + bo
    return out

